# revision 2
# baseline (speedup 1.0000x reference)
"""Chamfer loss kernel for Trainium2 (8 NeuronCores, Bass/Tile).

Problem: x (4, 8192, 3), y (4, 8192, 3) fp32.
  dist[b,i,j] = ||x_bi||^2 + ||y_bj||^2 - 2 x_bi . y_bj
  out = mean_b( mean_i min_j dist + mean_j min_i dist )

Sharding: 8 cores = 4 batches x 2 halves. Core (b, h) computes
  - x->y mins for x rows [h*4096, (h+1)*4096) of batch b vs ALL y[b]
  - y->x mins for y rows [h*4096, (h+1)*4096) of batch b vs ALL x[b]
so no cross-core reduction is needed (each core owns full rows of output).

On-chip compute: G[i,j] = -2 q_i . d_j + ||d_j||^2 via a single K=14 bf16
matmul using hi/lo splitting (full-fp32-class accuracy at bf16 matmul speed):
  q = A + AL (+ eps),  -2d = C + E (+ eps),  ||d||^2 = d2h + d2l (+ eps)
  G = A.C + A.E + AL.C + AL.E + d2h + d2l
Then min_j dist = ||q_i||^2 + min_j G[i,j]; the min runs on VectorE from PSUM.
The ||q_i||^2 add + means happen on the host in float64 (cheap: O(N)).
"""

import numpy as np
import ml_dtypes

B = 4
N = 8192  # x points per batch
M = 8192  # y points per batch
D = 3
NCORES = 8

QROWS = 4096  # query rows per core (half of a batch's points)
DBN = 8192  # database points scanned per query
KDIM = 14  # augmented contraction dim
BLKP = 128  # query rows per matmul block (PSUM partitions)
FREE = 512  # matmul free size (one PSUM fp32 bank)
GROUP = 2048  # PSUM group reduced by one DVE op (4 banks)

_NC_CACHE = {}


def _build_nc(qrows=QROWS, dbn=DBN):
    """Build + compile the (SPMD, identical on all cores) Bass program."""
    from contextlib import ExitStack

    import concourse.tile as tile
    from concourse import bacc, mybir

    bf16 = mybir.dt.bfloat16
    f32 = mybir.dt.float32

    nblk = qrows // BLKP
    ngrp = dbn // GROUP
    outc = nblk * ngrp

    nc = bacc.Bacc(
        "TRN2", target_bir_lowering=False, debug=False, num_devices=NCORES
    )
    lx = nc.dram_tensor("lx", [KDIM, qrows], bf16, kind="ExternalInput")
    ry = nc.dram_tensor("ry", [KDIM, dbn], bf16, kind="ExternalInput")
    ly = nc.dram_tensor("ly", [KDIM, qrows], bf16, kind="ExternalInput")
    rx = nc.dram_tensor("rx", [KDIM, dbn], bf16, kind="ExternalInput")
    ox = nc.dram_tensor("ox", [BLKP, outc], f32, kind="ExternalOutput")
    oy = nc.dram_tensor("oy", [BLKP, outc], f32, kind="ExternalOutput")

    with tile.TileContext(nc) as tc, ExitStack() as ctx:
        cpool = ctx.enter_context(tc.tile_pool(name="consts", bufs=1))
        ppool = ctx.enter_context(tc.tile_pool(name="psum", bufs=2, space="PSUM"))
        opool = ctx.enter_context(tc.tile_pool(name="outs", bufs=1))

        s_lx = cpool.tile([KDIM, qrows], bf16, tag="lx")
        s_ry = cpool.tile([KDIM, dbn], bf16, tag="ry")
        s_ly = cpool.tile([KDIM, qrows], bf16, tag="ly")
        s_rx = cpool.tile([KDIM, dbn], bf16, tag="rx")
        nc.sync.dma_start(s_lx[:], lx[:])
        nc.sync.dma_start(s_ry[:], ry[:])
        nc.sync.dma_start(s_ly[:], ly[:])
        nc.sync.dma_start(s_rx[:], rx[:])

        s_ox = opool.tile([BLKP, outc], f32, tag="ox")
        s_oy = opool.tile([BLKP, outc], f32, tag="oy")

        for s_l, s_r, s_o, o_dram in (
            (s_lx, s_ry, s_ox, ox),
            (s_ly, s_rx, s_oy, oy),
        ):
            for blk in range(nblk):
                lhs_blk = s_l[:, blk * BLKP : (blk + 1) * BLKP]
                for g in range(ngrp):
                    ps = ppool.tile([BLKP, GROUP], f32, tag="ps")
                    for t in range(GROUP // FREE):
                        col0 = g * GROUP + t * FREE
                        nc.tensor.matmul(
                            ps[:, t * FREE : (t + 1) * FREE],
                            lhs_blk,
                            s_r[:, col0 : col0 + FREE],
                            start=True,
                            stop=True,
                        )
                    oc = blk * ngrp + g
                    nc.vector.tensor_reduce(
                        s_o[:, oc : oc + 1],
                        ps[:],
                        axis=mybir.AxisListType.X,
                        op=mybir.AluOpType.min,
                    )
            nc.sync.dma_start(o_dram[:], s_o[:])

    nc.compile()
    return nc


def _get_nc(qrows=QROWS, dbn=DBN):
    key = (qrows, dbn)
    if key not in _NC_CACHE:
        _NC_CACHE[key] = _build_nc(qrows, dbn)
    return _NC_CACHE[key]


def _split_hi_lo(a):
    """fp32 array -> (hi, lo) bf16 pair with hi+lo ~ a (error ~2^-18 |a|)."""
    hi = a.astype(ml_dtypes.bfloat16)
    lo = (a - hi.astype(np.float32)).astype(ml_dtypes.bfloat16)
    return hi, lo


def _build_lhs(q):
    """q [Q, 3] fp32 -> stationary operand [14, Q] bf16."""
    qq = np.ascontiguousarray(q.T)  # [3, Q]
    A, AL = _split_hi_lo(qq)
    ones = np.ones((1, q.shape[0]), dtype=ml_dtypes.bfloat16)
    return np.concatenate([A, A, AL, AL, ones, ones], axis=0)


def _build_rhs(d):
    """d [Dn, 3] fp32 -> moving operand [14, Dn] bf16."""
    t = np.ascontiguousarray(d.T) * np.float32(-2.0)  # [3, Dn]
    C, E = _split_hi_lo(t)
    d2 = (d.astype(np.float64) ** 2).sum(axis=1).astype(np.float32)[None, :]
    d2h, d2l = _split_hi_lo(d2)
    return np.concatenate([C, E, C, E, d2h, d2l], axis=0)


def _unpack_mins(o, nblk, ngrp):
    """o [128, nblk*ngrp] fp32 per-group G-mins -> [nblk*128] row G-mins."""
    v = o.reshape(BLKP, nblk, ngrp).min(axis=2)  # [p, blk]
    return v.T.reshape(-1)  # row = blk*128 + p


def kernel(x, y):
    from concourse.bass_utils import run_bass_kernel_spmd

    x = np.asarray(x, dtype=np.float32)
    y = np.asarray(y, dtype=np.float32)
    assert x.shape == (B, N, D) and y.shape == (B, M, D)

    nblk = QROWS // BLKP
    ngrp = DBN // GROUP

    in_maps = []
    rhs_y = [_build_rhs(y[b]) for b in range(B)]
    rhs_x = [_build_rhs(x[b]) for b in range(B)]
    for c in range(NCORES):
        b, h = divmod(c, 2)
        sl = slice(h * QROWS, (h + 1) * QROWS)
        in_maps.append(
            {
                "lx": _build_lhs(x[b, sl]),
                "ry": rhs_y[b],
                "ly": _build_lhs(y[b, sl]),
                "rx": rhs_x[b],
            }
        )

    nc = _get_nc()
    res = run_bass_kernel_spmd(nc, in_maps, core_ids=list(range(NCORES)))

    total = 0.0
    for b in range(B):
        x2 = (x[b].astype(np.float64) ** 2).sum(axis=1)  # [N]
        y2 = (y[b].astype(np.float64) ** 2).sum(axis=1)  # [M]
        minx = np.empty(N, dtype=np.float64)
        miny = np.empty(M, dtype=np.float64)
        for h in range(2):
            r = res.results[2 * b + h]
            sl = slice(h * QROWS, (h + 1) * QROWS)
            minx[sl] = _unpack_mins(r["ox"], nblk, ngrp)
            miny[sl] = _unpack_mins(r["oy"], nblk, ngrp)
        minx += x2
        miny += y2
        total += minx.mean() + miny.mean()

    return np.float32(total / B)


# revision 5
# speedup vs baseline: 5.5765x; 5.5765x over previous
"""Chamfer loss kernel for Trainium2 (8 NeuronCores, Bass/Tile).

Problem: x (4, 8192, 3), y (4, 8192, 3) fp32.
  dist[b,i,j] = ||x_bi||^2 + ||y_bj||^2 - 2 x_bi . y_bj
  out = mean_b( mean_i min_j dist + mean_j min_i dist )

Sharding: 8 cores = 4 batches x 2 halves. Core (b, h) computes
  - x->y mins for x rows [h*4096, (h+1)*4096) of batch b vs ALL y[b]
  - y->x mins for y rows [h*4096, (h+1)*4096) of batch b vs ALL x[b]
so no cross-core reduction is needed (each core owns full rows of output).

On-chip compute: G[i,j] = -2 q_i . d_j + ||d_j||^2 via a single K=21 bf16
matmul using 3-term hi/mid/lo splitting (beyond-fp32 accuracy at bf16 matmul
speed; matmul cost depends only on the free dim, not K):
  q = A + AL + AL2 (+ 2^-27),  -2d = C + E + E2,  ||d||^2 = d2h + d2l + d2l2
  G = A.(C+E+E2) + AL.(C+E) + AL2.C + d2h + d2l + d2l2
Then min_j dist = ||q_i||^2 + min_j G[i,j]; the min runs on VectorE from PSUM.
The ||q_i||^2 add + means happen on the host in float64 (cheap: O(N)).
"""

import numpy as np
import ml_dtypes

B = 4
N = 8192  # x points per batch
M = 8192  # y points per batch
D = 3
NCORES = 8

QROWS = 4096  # query rows per core (half of a batch's points)
DBN = 8192  # database points scanned per query
KDIM = 21  # augmented contraction dim
BLKP = 128  # query rows per matmul block (PSUM partitions)
FREE = 512  # matmul free size (one PSUM fp32 bank)
GROUP = 2048  # PSUM group reduced by one DVE op (4 banks)

_NC_CACHE = {}


def _build_nc(qrows=QROWS, dbn=DBN):
    """Build + compile the (SPMD, identical on all cores) Bass program."""
    from contextlib import ExitStack

    import concourse.tile as tile
    from concourse import bacc, mybir

    bf16 = mybir.dt.bfloat16
    f32 = mybir.dt.float32

    nblk = qrows // BLKP
    ngrp = dbn // GROUP
    outc = nblk * ngrp

    nc = bacc.Bacc(
        "TRN2", target_bir_lowering=False, debug=False, num_devices=NCORES
    )
    lx = nc.dram_tensor("lx", [KDIM, qrows], bf16, kind="ExternalInput")
    ry = nc.dram_tensor("ry", [KDIM, dbn], bf16, kind="ExternalInput")
    ly = nc.dram_tensor("ly", [KDIM, qrows], bf16, kind="ExternalInput")
    rx = nc.dram_tensor("rx", [KDIM, dbn], bf16, kind="ExternalInput")
    ox = nc.dram_tensor("ox", [BLKP, outc], f32, kind="ExternalOutput")
    oy = nc.dram_tensor("oy", [BLKP, outc], f32, kind="ExternalOutput")

    with tile.TileContext(nc) as tc, ExitStack() as ctx:
        cpool = ctx.enter_context(tc.tile_pool(name="consts", bufs=1))
        ppool = ctx.enter_context(tc.tile_pool(name="psum", bufs=2, space="PSUM"))
        opool = ctx.enter_context(tc.tile_pool(name="outs", bufs=1))

        s_lx = cpool.tile([KDIM, qrows], bf16, tag="lx")
        s_ry = cpool.tile([KDIM, dbn], bf16, tag="ry")
        s_ly = cpool.tile([KDIM, qrows], bf16, tag="ly")
        s_rx = cpool.tile([KDIM, dbn], bf16, tag="rx")
        nc.sync.dma_start(s_lx[:], lx[:])
        nc.sync.dma_start(s_ry[:], ry[:])
        nc.sync.dma_start(s_ly[:], ly[:])
        nc.sync.dma_start(s_rx[:], rx[:])

        s_ox = opool.tile([BLKP, outc], f32, tag="ox")
        s_oy = opool.tile([BLKP, outc], f32, tag="oy")

        for s_l, s_r, s_o, o_dram in (
            (s_lx, s_ry, s_ox, ox),
            (s_ly, s_rx, s_oy, oy),
        ):
            for blk in range(nblk):
                lhs_blk = s_l[:, blk * BLKP : (blk + 1) * BLKP]
                for g in range(ngrp):
                    ps = ppool.tile([BLKP, GROUP], f32, tag="ps")
                    for t in range(GROUP // FREE):
                        col0 = g * GROUP + t * FREE
                        nc.tensor.matmul(
                            ps[:, t * FREE : (t + 1) * FREE],
                            lhs_blk,
                            s_r[:, col0 : col0 + FREE],
                            start=True,
                            stop=True,
                        )
                    oc = blk * ngrp + g
                    nc.vector.tensor_reduce(
                        s_o[:, oc : oc + 1],
                        ps[:],
                        axis=mybir.AxisListType.X,
                        op=mybir.AluOpType.min,
                    )
            nc.sync.dma_start(o_dram[:], s_o[:])

    nc.compile()
    return nc


def _get_nc(qrows=QROWS, dbn=DBN):
    key = (qrows, dbn)
    if key not in _NC_CACHE:
        _NC_CACHE[key] = _build_nc(qrows, dbn)
    return _NC_CACHE[key]


def _split3(a):
    """fp32 array -> (hi, mid, lo) bf16 triple, hi+mid+lo ~ a to ~2^-27 |a|."""
    hi = a.astype(ml_dtypes.bfloat16)
    r = a - hi.astype(np.float32)
    mid = r.astype(ml_dtypes.bfloat16)
    lo = (r - mid.astype(np.float32)).astype(ml_dtypes.bfloat16)
    return hi, mid, lo


def _build_lhs(q):
    """q [Q, 3] fp32 -> stationary operand [21, Q] bf16."""
    qq = np.ascontiguousarray(q.T)  # [3, Q]
    A, AL, AL2 = _split3(qq)
    ones = np.ones((3, q.shape[0]), dtype=ml_dtypes.bfloat16)
    return np.concatenate([A, A, A, AL, AL, AL2, ones], axis=0)


def _build_rhs(d):
    """d [Dn, 3] fp32 -> moving operand [21, Dn] bf16."""
    t = np.ascontiguousarray(d.T) * np.float32(-2.0)  # [3, Dn]
    C, E, E2 = _split3(t)
    d2 = (d.astype(np.float64) ** 2).sum(axis=1).astype(np.float32)[None, :]
    d2h, d2l, d2l2 = _split3(d2)
    return np.concatenate([C, E, E2, C, E, C, d2h, d2l, d2l2], axis=0)


def _unpack_mins(o, nblk, ngrp):
    """o [128, nblk*ngrp] fp32 per-group G-mins -> [nblk*128] row G-mins."""
    v = o.reshape(BLKP, nblk, ngrp).min(axis=2)  # [p, blk]
    return v.T.reshape(-1)  # row = blk*128 + p


def kernel(x, y):
    from concourse.bass_utils import run_bass_kernel_spmd

    x = np.asarray(x, dtype=np.float32)
    y = np.asarray(y, dtype=np.float32)
    assert x.shape == (B, N, D) and y.shape == (B, M, D)

    nblk = QROWS // BLKP
    ngrp = DBN // GROUP

    in_maps = []
    rhs_y = [_build_rhs(y[b]) for b in range(B)]
    rhs_x = [_build_rhs(x[b]) for b in range(B)]
    for c in range(NCORES):
        b, h = divmod(c, 2)
        sl = slice(h * QROWS, (h + 1) * QROWS)
        in_maps.append(
            {
                "lx": _build_lhs(x[b, sl]),
                "ry": rhs_y[b],
                "ly": _build_lhs(y[b, sl]),
                "rx": rhs_x[b],
            }
        )

    nc = _get_nc()
    res = run_bass_kernel_spmd(nc, in_maps, core_ids=list(range(NCORES)))

    total = 0.0
    for b in range(B):
        x2 = (x[b].astype(np.float64) ** 2).sum(axis=1)  # [N]
        y2 = (y[b].astype(np.float64) ** 2).sum(axis=1)  # [M]
        minx = np.empty(N, dtype=np.float64)
        miny = np.empty(M, dtype=np.float64)
        for h in range(2):
            r = res.results[2 * b + h]
            sl = slice(h * QROWS, (h + 1) * QROWS)
            minx[sl] = _unpack_mins(r["ox"], nblk, ngrp)
            miny[sl] = _unpack_mins(r["oy"], nblk, ngrp)
        minx += x2
        miny += y2
        total += minx.mean() + miny.mean()

    return np.float32(total / B)
